# revision 15
# baseline (speedup 1.0000x reference)
"""Expert-parallel grouped GEMM (MoE) kernel for Trainium2.

Problem: out[e] = gelu(tok[e] @ w1[e]) @ w2[e]  per expert e.
  tok: [128, 2048, 128] f32, w1: [128, 128, 512] f32, w2: [128, 512, 128] f32.

Sharding: expert-parallel across 8 NeuronCores, 16 experts per core, no
cross-core communication. Each core runs the same Bass program on its own
expert slice (SPMD), the host concatenates the per-core outputs.

v2 dataflow (per core, per 512-token chunk):
  - tokens loaded via casting SWDGE DMA straight to bf16, natural [t, d]
    blocks (partition = t within a 128-token block)
  - PE-transpose token blocks to tokT [d, t] (bf16, 1 cyc/row), DVE copies
    PSUM -> SBUF (2x mode)
  - MM1 (bf16): hT[hd, t] = w1b.T @ tokT, into pair PSUM tiles [128, 2, 512]
  - GELU pair ops on ScalarE: PSUM f32 -> SBUF bf16 ht tiles
  - MM2 (bf16): po[t, o] += ht[hd-slice, t-block].T @ w2b[hd-slice]
    -- ht slices act as the (transposed-consumed) stationary, so the output
    lands in natural [t, o] layout: no output transposes at all
  - Pool drains po PSUM -> SBUF f32, SP HWDGE stores natural [t, o]
  - weights: f32 via SP HWDGE, DVE-cast to bf16 per expert
"""

import numpy as np

NUM_CORES = 8
E_TOTAL = 128
E_PER_CORE = E_TOTAL // NUM_CORES  # 16
T = 2048
D = 128
H = 512
O = 128
P = 128

T_CHUNK = 512
N_CHUNKS = T // T_CHUNK  # 4
BLKS = T_CHUNK // P  # 4 token blocks per chunk
H_TILES = H // P  # 4

_CACHE = {}


DEFAULT_CFG = dict(
    tokb_bufs=3,
    tokc_bufs=4,
    tokt_bufs=3,
    ht_bufs=4,
    oc_bufs=4,
    w_bufs=2,
    pt_bufs=2,
    ph_bufs=2,
    po_bufs=2,
)


def _build(loop=1, cfg=None):
    import concourse.bacc as bacc
    import concourse.mybir as mybir
    import concourse.tile as tile
    from concourse.masks import make_identity

    f32 = mybir.dt.float32
    bf16 = mybir.dt.bfloat16
    GELU = mybir.ActivationFunctionType.Gelu
    C = dict(DEFAULT_CFG)
    if cfg:
        C.update(cfg)

    nc = bacc.Bacc(
        "TRN2",
        target_bir_lowering=False,
        debug=False,
        num_devices=NUM_CORES,
    )

    tok = nc.dram_tensor(
        "group_token", [E_PER_CORE, T, D], f32, kind="ExternalInput"
    ).ap()
    w1 = nc.dram_tensor("weights1", [E_PER_CORE, D, H], f32, kind="ExternalInput").ap()
    w2 = nc.dram_tensor("weights2", [E_PER_CORE, H, O], f32, kind="ExternalInput").ap()
    out = nc.dram_tensor("out", [E_PER_CORE, T, O], f32, kind="ExternalOutput").ap()

    with tile.TileContext(nc) as tc:
        with (
            tc.tile_pool(name="const", bufs=1) as const_pool,
            tc.tile_pool(name="wf", bufs=C["w_bufs"]) as wf_pool,
            tc.tile_pool(name="wb", bufs=C["w_bufs"]) as wb_pool,
            tc.tile_pool(name="tokb", bufs=C["tokb_bufs"]) as tokb_pool,
            tc.tile_pool(name="tokc", bufs=C["tokc_bufs"]) as tokc_pool,
            tc.tile_pool(name="tokt", bufs=C["tokt_bufs"]) as tokt_pool,
            tc.tile_pool(name="ht", bufs=C["ht_bufs"]) as ht_pool,
            tc.tile_pool(name="oc", bufs=C["oc_bufs"]) as oc_pool,
            tc.tile_pool(name="pt", bufs=C["pt_bufs"], space="PSUM") as pt_pool,
            tc.tile_pool(name="ph", bufs=C["ph_bufs"], space="PSUM") as ph_pool,
            tc.tile_pool(name="po", bufs=C["po_bufs"], space="PSUM") as po_pool,
        ):
            ident_f32 = const_pool.tile([P, P], f32)
            make_identity(nc, ident_f32)
            ident = const_pool.tile([P, P], bf16)
            nc.vector.tensor_copy(ident[:], ident_f32[:])

            NG = E_PER_CORE * N_CHUNKS  # 64 global chunks

            def body(_iv=None):
                state = {}  # e -> (w1b, w2b)
                tokstate = {}  # e -> token tiles (set at setup_dma time)
                fstate = {}  # e -> (w1f, w2f) until the bf16 casts are emitted

                def setup_dma(e):
                    # tokens: casting DMA (gpsimd SWDGE) f32 -> bf16, natural
                    # blocks: partition = t within block, block (c, j).
                    # Expert 0's chunk-0 goes first: it gates the whole pipe.
                    if e == 0:
                        toks = []
                        for c in range(N_CHUNKS):
                            tkc = tokc_pool.tile(
                                [P, BLKS, D], bf16, tag="tokc", name=f"tokc{c}"
                            )
                            nc.gpsimd.dma_start(
                                tkc[:],
                                tok[e].rearrange(
                                    "(c j p) d -> c p j d", c=N_CHUNKS, j=BLKS, p=P
                                )[c],
                            )
                            toks.append(tkc)
                    else:
                        tf = tokb_pool.tile(
                            [P, N_CHUNKS * BLKS, D], bf16, tag="tokb", name=f"tokb{e}"
                        )
                        nc.gpsimd.dma_start(
                            tf[:],
                            tok[e].rearrange("(m p) d -> p m d", p=P),
                        )
                        toks = tf
                    # weights f32 via SP HWDGE; bf16 casts emitted later
                    # (setup_cast) so they don't delay the critical tokt copy
                    w1f = wf_pool.tile([P, H], f32, tag="w1f", name=f"w1f{e}")
                    nc.sync.dma_start(w1f[:], w1[e])
                    w2f = wf_pool.tile([P, H_TILES, O], f32, tag="w2f", name=f"w2f{e}")
                    nc.sync.dma_start(w2f[:], w2[e].rearrange("(k p) o -> p k o", p=P))
                    fstate[e] = (w1f, w2f)
                    tokstate[e] = toks

                def setup_cast(e):
                    w1f, w2f = fstate.pop(e)
                    w1b = wb_pool.tile([P, H], bf16, tag="w1b", name=f"w1b{e}")
                    nc.vector.tensor_copy(w1b[:], w1f[:])
                    w2b = wb_pool.tile([P, H_TILES, O], bf16, tag="w2b", name=f"w2b{e}")
                    nc.vector.tensor_copy(w2b[:], w2f[:])
                    state[e] = (w1b, w2b)

                def blk(g, j):
                    e, c = divmod(g, N_CHUNKS)
                    toks = tokstate[e]
                    if isinstance(toks, list):
                        return toks[c][:, j]
                    return toks[:, c * BLKS + j]

                pts = {}
                hts = {}
                pos = {}

                def tin(g, j):
                    # PE transpose token block j of chunk g into pt[g]
                    if j == 0:
                        pts[g] = pt_pool.tile(
                            [P, T_CHUNK], bf16, tag="pt", name=f"pt{g}"
                        )
                    nc.tensor.transpose(
                        pts[g][:, j * P : (j + 1) * P], blk(g, j), ident[:]
                    )

                def mm2(g, b):
                    # po[:, b, :] += ht[hd].T @ w2b[hd] over 4 hd tiles
                    e = g // N_CHUNKS
                    w2b = state[e][1]
                    hta, htb = hts[g]
                    for k in range(H_TILES):
                        src = hta if k < 2 else htb
                        nc.tensor.matmul(
                            pos[g][:, b],
                            src[:, k % 2, b * P : (b + 1) * P],
                            w2b[:, k],
                            start=(k == 0),
                            stop=(k == H_TILES - 1),
                        )

                def drain(g, half=None):
                    e, c = divmod(g, N_CHUNKS)
                    dst = out[e].rearrange(
                        "(c b p) o -> c p b o", c=N_CHUNKS, p=P
                    )[c]
                    if half is None:
                        oc = oc_pool.tile([P, BLKS, O], f32, tag="oc", name=f"oc{g}")
                        nc.vector.tensor_copy(oc[:], pos.pop(g)[:])
                        nc.sync.dma_start(dst, oc[:])
                        hts.pop(g)
                        return
                    hb = BLKS // 2
                    sl = slice(half * hb, (half + 1) * hb)
                    oc = oc_pool.tile(
                        [P, hb, O], f32, tag="oc", name=f"oc{g}_{half}"
                    )
                    nc.vector.tensor_copy(oc[:], pos[g][:, sl])
                    nc.sync.dma_start(dst[:, sl], oc[:])
                    if half == 1:
                        pos.pop(g)
                        hts.pop(g)

                setup_dma(0)
                setup_cast(0)
                for j in range(BLKS):
                    tin(0, j)

                for g in range(NG):
                    e, c = divmod(g, N_CHUNKS)
                    if c == 2 and e + 1 < E_PER_CORE:
                        setup_dma(e + 1)
                    w1b = state[e][0]

                    tokt = tokt_pool.tile(
                        [P, T_CHUNK], bf16, tag="tokt", name=f"tokt{g}"
                    )
                    nc.vector.tensor_copy(tokt[:], pts.pop(g)[:])

                    # MM1 in two pair-PSUM tiles, GELU after each pair;
                    # next chunk's transposes go between the pairs so the
                    # tokt(g+1) chain starts as early as possible
                    pair_tiles = []
                    for hp in range(H_TILES // 2):
                        ph = ph_pool.tile(
                            [P, 2, T_CHUNK], f32, tag="ph", name=f"ph{g}_{hp}"
                        )
                        for k in range(2):
                            hd = hp * 2 + k
                            nc.tensor.matmul(
                                ph[:, k],
                                w1b[:, hd * P : (hd + 1) * P],
                                tokt[:],
                                start=True,
                                stop=True,
                            )
                        ht = ht_pool.tile(
                            [P, 2, T_CHUNK], bf16, tag="ht", name=f"ht{g}_{hp}"
                        )
                        act = (
                            (lambda o, i: nc.scalar.copy(o, i))
                            if C.get("gelu_copy")
                            else (lambda o, i: nc.scalar.activation(o, i, GELU))
                        )
                        if g == 0:
                            # startup: single-tile ops so Act starts sooner
                            act(ht[:, 0], ph[:, 0])
                            act(ht[:, 1], ph[:, 1])
                        else:
                            act(ht[:], ph[:])
                        pair_tiles.append(ht)
                        if hp == 0 and g + 1 < NG:
                            for j in range(BLKS):
                                tin(g + 1, j)
                    hts[g] = pair_tiles

                    # MM2 for previous chunk (its GELU finished last round)
                    if g >= 1:
                        pos[g - 1] = po_pool.tile(
                            [P, BLKS, O], f32, tag="po", name=f"po{g - 1}"
                        )
                        for b in range(BLKS):
                            mm2(g - 1, b)
                        drain(g - 1)
                    if c == 3 and e + 1 < E_PER_CORE:
                        setup_cast(e + 1)

                # tail: last chunk, drained in halves for an earlier store
                g = NG - 1
                pos[g] = po_pool.tile([P, BLKS, O], f32, tag="po", name=f"po{g}")
                mm2(g, 0)
                mm2(g, 1)
                drain(g, half=0)
                mm2(g, 2)
                mm2(g, 3)
                drain(g, half=1)

            if C.get("unroll"):
                for _ in range(int(C["unroll"])):
                    body()
            elif loop == 1:
                body()
            else:
                with tc.For_i(0, loop, 1) as _i:
                    body(_i)

    nc.compile()
    return nc


def _get_nc(loop=1, cfg=None):
    key = ("nc", loop, tuple(sorted((cfg or {}).items())))
    if key not in _CACHE:
        _CACHE[key] = _build(loop, cfg)
    return _CACHE[key]


def kernel(group_token, weights1, weights2):
    from concourse.bass_utils import run_bass_kernel_spmd

    group_token = np.ascontiguousarray(np.asarray(group_token, dtype=np.float32))
    weights1 = np.ascontiguousarray(np.asarray(weights1, dtype=np.float32))
    weights2 = np.ascontiguousarray(np.asarray(weights2, dtype=np.float32))

    nc = _get_nc()
    in_maps = []
    for c in range(NUM_CORES):
        sl = slice(c * E_PER_CORE, (c + 1) * E_PER_CORE)
        in_maps.append(
            {
                "group_token": np.ascontiguousarray(group_token[sl]),
                "weights1": np.ascontiguousarray(weights1[sl]),
                "weights2": np.ascontiguousarray(weights2[sl]),
            }
        )

    res = run_bass_kernel_spmd(nc, in_maps, core_ids=list(range(NUM_CORES)))
    _CACHE["last_results"] = res
    return np.concatenate([r["out"] for r in res.results], axis=0)


# revision 27
# speedup vs baseline: 1.1743x; 1.1743x over previous
"""Expert-parallel grouped GEMM (MoE) kernel for Trainium2.

Problem: out[e] = gelu(tok[e] @ w1[e]) @ w2[e]  per expert e.
  tok: [128, 2048, 128] f32, w1: [128, 128, 512] f32, w2: [128, 512, 128] f32.

Sharding: expert-parallel across 8 NeuronCores, 16 experts per core, no
cross-core communication. Each core runs the same Bass program on its own
expert slice (SPMD), the host concatenates the per-core outputs.

v2 dataflow (per core, per 512-token chunk):
  - tokens loaded via casting SWDGE DMA straight to bf16, natural [t, d]
    blocks (partition = t within a 128-token block)
  - PE-transpose token blocks to tokT [d, t] (bf16, 1 cyc/row), DVE copies
    PSUM -> SBUF (2x mode)
  - MM1 (bf16): hT[hd, t] = w1b.T @ tokT, into pair PSUM tiles [128, 2, 512]
  - GELU pair ops on ScalarE: PSUM f32 -> SBUF bf16 ht tiles
  - MM2 (bf16): po[t, o] += ht[hd-slice, t-block].T @ w2b[hd-slice]
    -- ht slices act as the (transposed-consumed) stationary, so the output
    lands in natural [t, o] layout: no output transposes at all
  - Pool drains po PSUM -> SBUF f32, SP HWDGE stores natural [t, o]
  - weights: f32 via SP HWDGE, DVE-cast to bf16 per expert
"""

import numpy as np

NUM_CORES = 8
E_TOTAL = 128
E_PER_CORE = E_TOTAL // NUM_CORES  # 16
T = 2048
D = 128
H = 512
O = 128
P = 128

T_CHUNK = 512
N_CHUNKS = T // T_CHUNK  # 4
BLKS = T_CHUNK // P  # 4 token blocks per chunk
H_TILES = H // P  # 4

_CACHE = {}


DEFAULT_CFG = dict(
    tokb_bufs=3,
    tokc_bufs=4,
    tokt_bufs=3,
    ht_bufs=4,
    oc_bufs=4,
    w_bufs=2,
    pt_bufs=2,
    ph_bufs=2,
    po_bufs=2,
)


def _build(loop=1, cfg=None):
    import concourse.bacc as bacc
    import concourse.mybir as mybir
    import concourse.tile as tile
    from concourse.masks import make_identity

    f32 = mybir.dt.float32
    bf16 = mybir.dt.bfloat16
    GELU = mybir.ActivationFunctionType.Gelu
    C = dict(DEFAULT_CFG)
    if cfg:
        C.update(cfg)

    nc = bacc.Bacc(
        "TRN2",
        target_bir_lowering=False,
        debug=False,
        num_devices=NUM_CORES,
    )

    tok = nc.dram_tensor(
        "group_token", [E_PER_CORE, T, D], f32, kind="ExternalInput"
    ).ap()
    w1 = nc.dram_tensor("weights1", [E_PER_CORE, D, H], f32, kind="ExternalInput").ap()
    w2 = nc.dram_tensor("weights2", [E_PER_CORE, H, O], f32, kind="ExternalInput").ap()
    out = nc.dram_tensor("out", [E_PER_CORE, T, O], f32, kind="ExternalOutput").ap()

    with tile.TileContext(nc) as tc:
        with (
            tc.tile_pool(name="const", bufs=1) as const_pool,
            tc.tile_pool(name="wf", bufs=C["w_bufs"]) as wf_pool,
            tc.tile_pool(name="wb", bufs=C["w_bufs"]) as wb_pool,
            tc.tile_pool(name="tokb", bufs=C["tokb_bufs"]) as tokb_pool,
            tc.tile_pool(name="tokc", bufs=C["tokc_bufs"]) as tokc_pool,
            tc.tile_pool(name="tokt", bufs=C["tokt_bufs"]) as tokt_pool,
            tc.tile_pool(name="ht", bufs=C["ht_bufs"]) as ht_pool,
            tc.tile_pool(name="oc", bufs=C["oc_bufs"]) as oc_pool,
            tc.tile_pool(name="pt", bufs=C["pt_bufs"], space="PSUM") as pt_pool,
            tc.tile_pool(name="ph", bufs=C["ph_bufs"], space="PSUM") as ph_pool,
            tc.tile_pool(name="po", bufs=C["po_bufs"], space="PSUM") as po_pool,
        ):
            ident_f32 = const_pool.tile([P, P], f32)
            make_identity(nc, ident_f32)
            ident = const_pool.tile([P, P], bf16)
            nc.vector.tensor_copy(ident[:], ident_f32[:])

            NG = E_PER_CORE * N_CHUNKS  # 64 global chunks

            def body(_iv=None):
                state = {}  # e -> (w1b, w2b)
                tokstate = {}  # e -> token tiles (set at setup_dma time)
                fstate = {}  # e -> (w1f, w2f) until the bf16 casts are emitted

                def setup_dma(e):
                    # tokens: casting DMA (gpsimd SWDGE) f32 -> bf16, natural
                    # blocks: partition = t within block, block (c, j).
                    # Expert 0's chunk-0 goes first: it gates the whole pipe.
                    if e == 0:
                        toks = []
                        for c in range(N_CHUNKS):
                            tkc = tokc_pool.tile(
                                [P, BLKS, D], bf16, tag="tokc", name=f"tokc{c}"
                            )
                            nc.gpsimd.dma_start(
                                tkc[:],
                                tok[e].rearrange(
                                    "(c j p) d -> c p j d", c=N_CHUNKS, j=BLKS, p=P
                                )[c],
                            )
                            toks.append(tkc)
                    else:
                        tf = tokb_pool.tile(
                            [P, N_CHUNKS * BLKS, D], bf16, tag="tokb", name=f"tokb{e}"
                        )
                        nc.gpsimd.dma_start(
                            tf[:],
                            tok[e].rearrange("(m p) d -> p m d", p=P),
                        )
                        toks = tf
                    # weights f32 via SP HWDGE; bf16 casts emitted later
                    # (setup_cast) so they don't delay the critical tokt copy
                    w1f = wf_pool.tile([P, H], f32, tag="w1f", name=f"w1f{e}")
                    nc.sync.dma_start(w1f[:], w1[e])
                    w2f = wf_pool.tile([P, H_TILES, O], f32, tag="w2f", name=f"w2f{e}")
                    nc.sync.dma_start(w2f[:], w2[e].rearrange("(k p) o -> p k o", p=P))
                    fstate[e] = (w1f, w2f)
                    tokstate[e] = toks

                def setup_cast(e):
                    w1f, w2f = fstate.pop(e)
                    w1b = wb_pool.tile([P, H], bf16, tag="w1b", name=f"w1b{e}")
                    nc.vector.tensor_copy(w1b[:], w1f[:])
                    w2b = wb_pool.tile([P, H_TILES, O], bf16, tag="w2b", name=f"w2b{e}")
                    nc.vector.tensor_copy(w2b[:], w2f[:])
                    state[e] = (w1b, w2b)

                def blk(g, j):
                    e, c = divmod(g, N_CHUNKS)
                    toks = tokstate[e]
                    if isinstance(toks, list):
                        return toks[c][:, j]
                    return toks[:, c * BLKS + j]

                pts = {}
                hts = {}
                pos = {}

                def tin(g, j):
                    # PE transpose token block j of chunk g into pt[g]
                    if j == 0:
                        pts[g] = pt_pool.tile(
                            [P, T_CHUNK], bf16, tag="pt", name=f"pt{g}"
                        )
                    for _rep in range(2 if C.get("double_tin") else 1):
                        nc.tensor.transpose(
                            pts[g][:, j * P : (j + 1) * P], blk(g, j), ident[:]
                        )

                def mm2(g, b):
                    # po[:, b, :] += ht[hd].T @ w2b[hd] over 4 hd tiles
                    e = g // N_CHUNKS
                    w2b = state[e][1]
                    hta, htb = hts[g]
                    for _rep in range(2 if C.get("double_mm2") else 1):
                        for k in range(H_TILES):
                            src = hta if k < 2 else htb
                            nc.tensor.matmul(
                                pos[g][:, b],
                                src[:, k % 2, b * P : (b + 1) * P],
                                w2b[:, k],
                                start=(k == 0),
                                stop=(k == H_TILES - 1),
                            )

                def drain(g, half=None):
                    e, c = divmod(g, N_CHUNKS)
                    dst = out[e].rearrange(
                        "(c b p) o -> c p b o", c=N_CHUNKS, p=P
                    )[c]
                    if half is None:
                        oc = oc_pool.tile([P, BLKS, O], f32, tag="oc", name=f"oc{g}")
                        nc.vector.tensor_copy(oc[:], pos.pop(g)[:])
                        nc.sync.dma_start(dst, oc[:])
                        hts.pop(g)
                        return
                    hb = BLKS // 2
                    sl = slice(half * hb, (half + 1) * hb)
                    oc = oc_pool.tile(
                        [P, hb, O], f32, tag="oc", name=f"oc{g}_{half}"
                    )
                    nc.vector.tensor_copy(oc[:], pos[g][:, sl])
                    nc.sync.dma_start(dst[:, sl], oc[:])
                    if half == 1:
                        pos.pop(g)
                        hts.pop(g)

                setup_dma(0)
                setup_cast(0)
                for j in range(BLKS):
                    tin(0, j)

                for g in range(NG):
                    e, c = divmod(g, N_CHUNKS)
                    if c == 2 and e + 1 < E_PER_CORE:
                        setup_dma(e + 1)
                    w1b = state[e][0]

                    tokt = tokt_pool.tile(
                        [P, T_CHUNK], bf16, tag="tokt", name=f"tokt{g}"
                    )
                    nc.vector.tensor_copy(tokt[:], pts.pop(g)[:])

                    # MM1 in two pair-PSUM tiles, GELU after each pair;
                    # next chunk's transposes go between the pairs so the
                    # tokt(g+1) chain starts as early as possible
                    pair_tiles = []
                    if C.get("skip_act"):
                        # timing ablation: MM2 reads a dummy ht, Act idle
                        if "ht_dummy" not in state:
                            dummy = ht_pool.tile(
                                [P, 2, T_CHUNK], bf16, tag="htd", name="ht_dummy"
                            )
                            nc.vector.memset(dummy[:], 0.5)
                            state["ht_dummy"] = dummy
                        dummy = state["ht_dummy"]
                    for hp in range(H_TILES // 2):
                        ph = ph_pool.tile(
                            [P, 2, T_CHUNK], f32, tag="ph", name=f"ph{g}_{hp}"
                        )
                        for _rep in range(2 if C.get("double_mm1") else 1):
                            for k in range(2):
                                hd = hp * 2 + k
                                nc.tensor.matmul(
                                    ph[:, k],
                                    w1b[:, hd * P : (hd + 1) * P],
                                    tokt[:],
                                    start=True,
                                    stop=True,
                                )
                        if C.get("skip_act"):
                            pair_tiles.append(dummy)
                        else:
                            ht = ht_pool.tile(
                                [P, 2, T_CHUNK], bf16, tag="ht", name=f"ht{g}_{hp}"
                            )
                            act = (
                                (lambda o, i: nc.scalar.copy(o, i))
                                if C.get("gelu_copy")
                                else (lambda o, i: nc.scalar.activation(o, i, GELU))
                            )
                            if g == 0:
                                # startup: single-tile ops so Act starts sooner
                                act(ht[:, 0], ph[:, 0])
                                act(ht[:, 1], ph[:, 1])
                            else:
                                act(ht[:], ph[:])
                            if C.get("double_act"):
                                sink = ht_pool.tile(
                                    [P, 2, T_CHUNK],
                                    bf16,
                                    tag="hts",
                                    name=f"htsink{g}_{hp}",
                                )
                                act(sink[:], ph[:])
                            pair_tiles.append(ht)
                        if hp == 0 and g + 1 < NG:
                            for j in range(BLKS):
                                tin(g + 1, j)
                    hts[g] = pair_tiles

                    # MM2 for previous chunk (its GELU finished last round)
                    if g >= 1:
                        pos[g - 1] = po_pool.tile(
                            [P, BLKS, O], f32, tag="po", name=f"po{g - 1}"
                        )
                        for b in range(BLKS):
                            mm2(g - 1, b)
                        drain(g - 1)
                    if c == 3 and e + 1 < E_PER_CORE:
                        setup_cast(e + 1)

                # tail: last chunk, drained in halves for an earlier store
                g = NG - 1
                pos[g] = po_pool.tile([P, BLKS, O], f32, tag="po", name=f"po{g}")
                mm2(g, 0)
                mm2(g, 1)
                drain(g, half=0)
                mm2(g, 2)
                mm2(g, 3)
                drain(g, half=1)

            if C.get("unroll"):
                for _ in range(int(C["unroll"])):
                    body()
            elif loop == 1:
                body()
            else:
                with tc.For_i(0, loop, 1) as _i:
                    body(_i)

    nc.compile()
    return nc


def _get_nc(loop=1, cfg=None):
    key = ("nc", loop, tuple(sorted((cfg or {}).items())))
    if key not in _CACHE:
        _CACHE[key] = _build(loop, cfg)
    return _CACHE[key]


def kernel(group_token, weights1, weights2):
    from concourse.bass_utils import run_bass_kernel_spmd

    group_token = np.ascontiguousarray(np.asarray(group_token, dtype=np.float32))
    weights1 = np.ascontiguousarray(np.asarray(weights1, dtype=np.float32))
    weights2 = np.ascontiguousarray(np.asarray(weights2, dtype=np.float32))

    nc = _get_nc()
    in_maps = []
    for c in range(NUM_CORES):
        sl = slice(c * E_PER_CORE, (c + 1) * E_PER_CORE)
        in_maps.append(
            {
                "group_token": np.ascontiguousarray(group_token[sl]),
                "weights1": np.ascontiguousarray(weights1[sl]),
                "weights2": np.ascontiguousarray(weights2[sl]),
            }
        )

    res = run_bass_kernel_spmd(nc, in_maps, core_ids=list(range(NUM_CORES)))
    _CACHE["last_results"] = res
    return np.concatenate([r["out"] for r in res.results], axis=0)
